# revision 1
# baseline (speedup 1.0000x reference)
"""Trainium2 Bass kernel for CombinedLoss (dice + hausdorff), 8-core SPMD.

Sharding: batch B=32 -> 4 samples/core, 12 (b,c) pairs per core.
Device computes, per (b,c):
    P = x @ y^T - 0.5*y2[j]        (PE, float32r matmuls + K=1 append matmul)
    row:  max_j P -> d_xy^2 = max_i (x2[i] - 2*max_j P[i,:])     (DVE)
    col:  Q = P - 0.5*x2[i] (ACT bias); d_yx^2 = -2*min_j max_i Q (GPSIMD)
Host does: input transposes (layout prep), x2/y2 row norms, dice term,
final sqrt/max/mean combine.
"""

import numpy as np

import concourse.bass as bass
import concourse.bacc as bacc
import concourse.mybir as mybir
import concourse.tile as tile
import concourse.bass_isa as bass_isa
from concourse.bass_utils import run_bass_kernel_spmd
from bass_rust import AxisListType

B, C, H, W = 32, 3, 512, 512
NCORES = 8
BPC = B // NCORES           # samples per core
NBC = BPC * C               # (b,c) pairs per core = 12
WEIGHT_DICE = 0.4
WEIGHT_HAUSDORFF = 0.6
SMOOTH = 1e-05

f32 = mybir.dt.float32
f32r = mybir.dt.float32r
ALU = mybir.AluOpType
ACTF = mybir.ActivationFunctionType

_CACHE = {}
_UID = [0]


def _uid():
    _UID[0] += 1
    return _UID[0]


def _esel():
    e = np.zeros((NBC, NBC * 128), dtype=np.float32)
    for bc in range(NBC):
        e[bc, 128 * bc : 128 * bc + 128] = 1.0
    return e


def _build(repeat=1):
    nc = bacc.Bacc(None)
    # x and y stacked: xyt[bc, 0] = x^T, xyt[bc, 1] = y^T  (both w-major)
    xyt_d = nc.dram_tensor("xyt", [NBC, 2, W, H], f32r, kind="ExternalInput")
    # -0.5*x2 arranged [p, 4*bc+rb] so that i = 128*rb + p
    x2n_d = nc.dram_tensor("x2n", [128, NBC * 4], f32, kind="ExternalInput")
    # -0.5*y2: partition bc holds the y2 row for that bc
    y2n_d = nc.dram_tensor("y2n", [NBC, H], f32r, kind="ExternalInput")
    # one-hot selector: esel[k, 128*bc + i] = (k == bc), so
    # esel[:, 128*bc:+128].T @ y2n broadcasts y2 row bc over all partitions
    esel_d = nc.dram_tensor("esel", [NBC, NBC * 128], f32r, kind="ExternalInput")
    id_d = nc.dram_tensor("ident", [128, 128], f32, kind="ExternalInput")
    # res[:, 0:NBC] = rrow (row path), res[:, NBC:2*NBC] = rcol (col path)
    res_d = nc.dram_tensor("res", [128, 2 * NBC], f32, kind="ExternalOutput")

    with tile.TileContext(nc) as tc:
        with (
            tc.tile_pool(name="const", bufs=1) as cpool,
            tc.tile_pool(name="xy", bufs=3) as xypool,
            tc.tile_pool(name="q", bufs=2) as qpool,
            tc.tile_pool(name="small", bufs=2) as spool,
            tc.tile_pool(name="psum", bufs=1, space="PSUM") as ppool,
            tc.tile_pool(name="psumt", bufs=2, space="PSUM") as tpool,
        ):
            # prefetch bc0's first input chunk before the small constant
            # loads so the big stream starts immediately
            xyts0 = xypool.tile([128, 2 * 4 * H], f32r, tag="xyts", name="xyts_pre")
            nc.sync.dma_start(
                xyts0[:].rearrange("p (t wb i) -> p t wb i", t=2, wb=4)[:, :, 0, :],
                xyt_d[0, :, 0:128, :].rearrange("t p i -> p t i"),
            )
            x2n = cpool.tile([128, NBC * 4], f32, tag="x2n")
            nc.sync.dma_start(x2n[:], x2n_d[:])
            y2n = cpool.tile([NBC, H], f32r, tag="y2n")
            nc.sync.dma_start(y2n[:], y2n_d[:])
            esel = cpool.tile([NBC, NBC * 128], f32r, tag="esel")
            nc.sync.dma_start(esel[:], esel_d[:])
            ident = cpool.tile([128, 128], f32, tag="ident")
            nc.sync.dma_start(ident[:], id_d[:])
            res = cpool.tile([128, 2 * NBC], f32, tag="res")
            rrow = res[:, 0:NBC]
            rcol = res[:, NBC : 2 * NBC]

            for bc in [b for _ in range(repeat) for b in range(NBC)]:
                if bc == 0:
                    xyts = xyts0
                else:
                    xyts = xypool.tile([128, 2 * 4 * H], f32r, tag="xyts")
                # xyts[p, 2048*t + 512*wb + i] = (x if t==0 else y)^T[128*wb + p, i]
                # one DMA per wb chunk so wb=0 matmuls start after ~1MB
                for wb in range(1 if bc == 0 else 0, 4):
                    nc.sync.dma_start(
                        xyts[:].rearrange("p (t wb i) -> p t wb i", t=2, wb=4)[
                            :, :, wb, :
                        ],
                        xyt_d[bc, :, 128 * wb : 128 * wb + 128, :].rearrange(
                            "t p i -> p t i"
                        ),
                    )
                xts = xyts[:, 0 : 4 * H]
                yts = xyts[:, 4 * H : 8 * H]
                pm = spool.tile([128, 4], f32, tag="pm")
                qall = qpool.tile([128, 4 * H], f32, tag="qall")
                # P_rb = G_rb - 0.5*y2[j]; wb-outer emission so only the
                # final-wb matmuls trail the last input chunk; the y2 append
                # rides just after wb0 (it only needs y2n, on-chip from t=0).
                Ps = [ppool.tile([128, H], f32, tag=f"P{i}", name=f"P{_uid()}_{i}") for i in range(4)]
                for wb in range(4):
                    for rb in range(4):
                        lo = 512 * wb + 128 * rb
                        nc.tensor.matmul(
                            Ps[rb][:],
                            xts[:, lo : lo + 128],
                            yts[:, 512 * wb : 512 * wb + 512],
                            start=(wb == 0),
                            stop=(wb == 3),
                        )
                        if wb == 0:
                            nc.tensor.matmul(
                                Ps[rb][:],
                                esel[:, 128 * bc : 128 * bc + 128],
                                y2n[:],
                                start=False,
                                stop=False,
                            )
                qm2 = qpool.tile([128, H], f32, tag="qm2")
                for rb in range(4):
                    # qall = P + (-0.5*x2[i]) = -0.5*d2   (sole PSUM reader)
                    nc.scalar.activation(
                        qall[:, H * rb : H * rb + H],
                        Ps[rb][:],
                        ACTF.Identity,
                        bias=x2n[:, 4 * bc + rb : 4 * bc + rb + 1],
                        scale=1.0,
                    )
                    # row path: pm[:, rb] = max_j Q_rb
                    nc.vector.reduce_max(
                        pm[:, rb : rb + 1],
                        qall[:, H * rb : H * rb + H],
                        axis=AxisListType.X,
                    )
                    # col path: incremental max over rb blocks
                    if rb == 1:
                        nc.vector.tensor_tensor(
                            qm2[:], qall[:, 0:H], qall[:, H : 2 * H], op=ALU.max
                        )
                    elif rb > 1:
                        nc.vector.tensor_tensor(
                            qm2[:],
                            qm2[:],
                            qall[:, H * rb : H * rb + H],
                            op=ALU.max,
                        )
                # row: d_xy^2 = -2 * min_i max_j qall
                nc.vector.tensor_reduce(
                    rrow[:, bc : bc + 1], pm[:], axis=AxisListType.X, op=ALU.min
                )
                # partition-axis max via PE transpose: T[jm, 128c+p] = qm2[p, 128c+jm]
                T = tpool.tile([128, H], f32, tag="T", name=f"T{_uid()}")
                for c4 in range(4):
                    nc.tensor.transpose(
                        T[:, 128 * c4 : 128 * c4 + 128],
                        qm2[:, 128 * c4 : 128 * c4 + 128],
                        ident[:],
                    )
                # max over p (innermost of free), then min over c; min over jm on host
                mt = spool.tile([128, 4], f32, tag="mt")
                nc.vector.tensor_reduce(
                    mt[:],
                    T[:].rearrange("jm (c p) -> jm c p", c=4),
                    axis=AxisListType.X,
                    op=ALU.max,
                )
                nc.vector.tensor_reduce(
                    rcol[:, bc : bc + 1], mt[:], axis=AxisListType.X, op=ALU.min
                )
            nc.sync.dma_start(res_d[:], res[:])
    nc.finalize()
    return nc


def kernel(input, target, _stats=None):
    x = np.asarray(input, dtype=np.float32)
    y = np.asarray(target, dtype=np.float32)

    # ---- host: dice term ----
    xf = x.reshape(B, -1).astype(np.float64)
    yf = y.reshape(B, -1).astype(np.float64)
    inter = (xf * yf).sum(axis=1)
    union = xf.sum(axis=1) + yf.sum(axis=1)
    dice = float(np.mean(1.0 - (2.0 * inter + SMOOTH) / (union + SMOOTH)))

    # ---- host: layout prep for device ----
    xt = np.ascontiguousarray(x.transpose(0, 1, 3, 2))  # [B,C,W,H]
    yt = np.ascontiguousarray(y.transpose(0, 1, 3, 2))
    x2 = (x.astype(np.float64) ** 2).sum(axis=-1).astype(np.float32)  # [B,C,H]
    y2 = (y.astype(np.float64) ** 2).sum(axis=-1).astype(np.float32)

    in_maps = []
    for c in range(NCORES):
        b0 = c * BPC
        x2c = (-0.5 * x2[b0 : b0 + BPC]).reshape(NBC, 4, 128)
        x2n = np.ascontiguousarray(x2c.transpose(2, 0, 1)).reshape(128, NBC * 4)
        y2n = (-0.5 * y2[b0 : b0 + BPC]).reshape(NBC, H)
        xyt = np.stack(
            [
                xt[b0 : b0 + BPC].reshape(NBC, W, H),
                yt[b0 : b0 + BPC].reshape(NBC, W, H),
            ],
            axis=1,
        )
        in_maps.append(
            {
                "xyt": np.ascontiguousarray(xyt),
                "x2n": np.ascontiguousarray(x2n),
                "y2n": np.ascontiguousarray(y2n),
                "ident": np.eye(128, dtype=np.float32),
                "esel": _esel(),
            }
        )

    if "nc" not in _CACHE:
        _CACHE["nc"] = _build()
    nc = _CACHE["nc"]

    import time as _time

    t0 = _time.time()
    br = run_bass_kernel_spmd(nc, in_maps, list(range(NCORES)), trace=False)
    t1 = _time.time()
    if isinstance(_stats, dict):
        _stats["wall_s"] = t1 - t0
        reps = _stats.get("repeats", 0)
        times = []
        for _ in range(reps):
            ta = _time.time()
            br = run_bass_kernel_spmd(nc, in_maps, list(range(NCORES)), trace=False)
            times.append(_time.time() - ta)
        _stats["repeat_wall_s"] = times

    # ---- host: combine ----
    hds = []
    for c in range(NCORES):
        res = br.results[c]["res"]  # [128, 2*NBC]
        rrow = res[:, :NBC]
        rcol = res[:, NBC:]
        dxy2 = -2.0 * rrow.min(axis=0)
        dyx2 = -2.0 * rcol.min(axis=0)
        hd2 = np.maximum(np.maximum(dxy2, dyx2), 0.0)
        hds.append(np.sqrt(hd2))
    hd = float(np.mean(np.concatenate(hds)))

    loss = WEIGHT_DICE * dice + WEIGHT_HAUSDORFF * hd
    return np.float32(loss)



# revision 9
# speedup vs baseline: 1.1453x; 1.1453x over previous
"""Trainium2 Bass kernel for CombinedLoss (dice + hausdorff), 8-core SPMD.

v2: fp8 DoubleRow matmul pipeline.

Sharding: batch B=32 -> 4 samples/core, NBC=12 (b,c) pairs per core.

Device computes, per (b,c), the full Gram Q = -0.5*d2(i,j) in PSUM via
fp8e4 DoubleRow matmuls (0.5 cyc/row): 8 main matmuls (K=2x128 each) plus
one augmented DoubleRow matmul per row-block that folds BOTH -0.5*x2[i]
and -0.5*y2[j] into the contraction as 6 extra K-dims (each norm encoded
as an fp8 hi/lo/lo2 triple against an all-ones counterpart, ~1e-2 exact).

Post-matmul per (b,c):
  ACT   drains P[:, :SPLIT] -> Q fp16 (single instruction)
  Pool  drains P[:, SPLIT:] (stt max-copy) -> Q fp16
  DVE   row path:  pairwise-max tree over j (scalar_tensor_tensor, 4x)
        -> pm[128,4] -> res[:, bc] = min over row-blocks
  DVE   col path:  2-step pairwise max over row-blocks -> qm2 [128,512]
  Pool  partition max (tensor_reduce axis=C) -> mall[bc, :]
Final: one DVE min over mall free axis -> res[0:12, 15], DMA res out.

Host does: fp8 rounding + layout packing, norms of the ROUNDED inputs
(so the device computes exact distances of a perturbed point set), dice
term (exact fp64), and the final sqrt/max/mean combine.
"""

import numpy as np
import ml_dtypes

import concourse.bass as bass
import concourse.bacc as bacc
import concourse.mybir as mybir
import concourse.tile as tile
from concourse.bass_utils import run_bass_kernel_spmd
from bass_rust import AxisListType

B, C, H, W = 32, 3, 512, 512
NCORES = 8
BPC = B // NCORES           # samples per core
NBC = BPC * C               # (b,c) pairs per core = 12
WEIGHT_DICE = 0.4
WEIGHT_HAUSDORFF = 0.6
SMOOTH = 1e-05

f32 = mybir.dt.float32
f16 = mybir.dt.float16
f8 = mybir.dt.float8e4
np8 = ml_dtypes.float8_e4m3
ALU = mybir.AluOpType
ACTF = mybir.ActivationFunctionType
DR = mybir.MatmulPerfMode.DoubleRow

# ACT drains P[:, :SPLIT]; Pool drains the rest (tune for engine balance)
SPLIT = 1792

_CACHE = {}


def _build():
    nc = bacc.Bacc(None)
    # xy8[bc, s, p, kb*1024 + t*512 + n] = (x if s==0 else y)[n, w],
    # w = kb*256 + t*128 + p  (fp8-rounded, contiguous 2KB partition lines)
    xy8_d = nc.dram_tensor("xy8", [NBC, 2, 128, 2048], f8, kind="ExternalInput")
    # aug K-dims (g, t): (0,0)(0,1)(1,0) = x2 hi/lo/lo2 | ones
    #                    (1,1)(2,0)(2,1) = ones | y2 hi/lo/lo2
    # xa8[g, (bc*4 + rb)*256 + t*128 + i], ya8[g, bc*1024 + t*512 + j]
    xa_d = nc.dram_tensor("xa8", [3, NBC * 4 * 256], f8, kind="ExternalInput")
    ya_d = nc.dram_tensor("ya8", [3, NBC * 1024], f8, kind="ExternalInput")
    # res[:, 0:NBC] = rrow (row path); mr[0, 0:NBC] = col path scalars
    res_d = nc.dram_tensor("res", [128, 16], f32, kind="ExternalOutput")
    mr_d = nc.dram_tensor("mr", [1, 16], f32, kind="ExternalOutput")

    with tile.TileContext(nc) as tc:
        with (
            tc.tile_pool(name="const", bufs=1) as cpool,
            tc.tile_pool(name="xy", bufs=3) as xypool,
            tc.tile_pool(name="q", bufs=2) as qpool,
            tc.tile_pool(name="wk", bufs=2) as wpool,
            tc.tile_pool(name="psum", bufs=2, space="PSUM") as ppool,
        ):
            # prefetch bc0's inputs before the small constant loads
            xy0 = xypool.tile([128, 4096], f8, tag="xy", name="xy_pre")
            nc.sync.dma_start(
                xy0[:].rearrange("p (s c) -> p s c", s=2),
                xy8_d[0].rearrange("s p c -> p s c"),
            )
            xa = cpool.tile([3, NBC * 4 * 256], f8, tag="xa")
            nc.sync.dma_start(xa[:], xa_d[:])
            ya = cpool.tile([3, NBC * 1024], f8, tag="ya")
            nc.sync.dma_start(ya[:], ya_d[:])
            mrow = cpool.tile([1, 16], f32, tag="mrow")
            res = cpool.tile([128, 16], f32, tag="res")
            minf = cpool.tile([1, 512], f16, tag="minf")
            nc.vector.memset(minf[:], -60000.0)

            for bc in range(NBC):
                if bc == 0:
                    xy = xy0
                else:
                    xy = xypool.tile([128, 4096], f8, tag="xy", name=f"xy{bc}")
                    nc.sync.dma_start(
                        xy[:].rearrange("p (s c) -> p s c", s=2),
                        xy8_d[bc].rearrange("s p c -> p s c"),
                    )
                xs = xy[:, 0:2048].rearrange("p (kb t n) -> p kb t n", kb=2, t=2)
                ys = xy[:, 2048:4096].rearrange("p (kb t n) -> p kb t n", kb=2, t=2)
                rhsa = ya[:, 1024 * bc : 1024 * bc + 1024].rearrange(
                    "g (t j) -> g t j", t=2
                )

                P = ppool.tile([128, 2048], f32, tag="P", name=f"P{bc}")
                for rb in range(4):
                    Prb = P[:, 512 * rb : 512 * rb + 512]
                    for kb in range(2):
                        nc.tensor.matmul(
                            Prb,
                            xs[:, kb, :, 128 * rb : 128 * rb + 128],
                            ys[:, kb],
                            start=(kb == 0),
                            stop=False,
                            perf_mode=DR,
                        )
                    o = (4 * bc + rb) * 256
                    nc.tensor.matmul(
                        Prb,
                        xa[:, o : o + 256].rearrange("g (t i) -> g t i", t=2),
                        rhsa,
                        start=False,
                        stop=True,
                        perf_mode=DR,
                    )

                # drain PSUM -> Q fp16 (one ACT instruction; GPSIMD can't
                # read PSUM, DVE fp32 reads are slower than ACT)
                Q = qpool.tile([128, 2048], f16, tag="Q", name=f"Q{bc}")
                nc.scalar.activation(Q[:], P[:], ACTF.Copy)

                # row path: per-rb max over j via pairwise-max tree (4x stt)
                v4 = Q[:].rearrange("p (rb j) -> p rb j", rb=4)
                t1 = wpool.tile([128, 1024], f16, tag="t1", name=f"t1_{bc}")
                nc.vector.scalar_tensor_tensor(
                    t1[:].rearrange("p (rb j) -> p rb j", rb=4),
                    v4[:, :, 0:256],
                    1.0,
                    v4[:, :, 256:512],
                    op0=ALU.mult,
                    op1=ALU.max,
                )
                src = t1
                w = 128
                while w >= 16:
                    dst = wpool.tile(
                        [128, 4 * w], f16, tag=f"t{w}", name=f"t{w}_{bc}"
                    )
                    sv = src[:, 0 : 8 * w].rearrange("p (rb j) -> p rb j", rb=4)
                    nc.vector.scalar_tensor_tensor(
                        dst[:].rearrange("p (rb j) -> p rb j", rb=4),
                        sv[:, :, 0:w],
                        1.0,
                        sv[:, :, w : 2 * w],
                        op0=ALU.mult,
                        op1=ALU.max,
                    )
                    src = dst
                    w //= 2
                # src = [128, (4, 16)] -> pm [128, 4] -> rrow col bc
                pm = wpool.tile([128, 4], f32, tag="pm", name=f"pm{bc}")
                nc.vector.tensor_reduce(
                    pm[:],
                    src[:].rearrange("p (rb j) -> p rb j", rb=4),
                    axis=AxisListType.X,
                    op=ALU.max,
                )
                nc.vector.tensor_reduce(
                    res[:, bc : bc + 1], pm[:], axis=AxisListType.X, op=ALU.min
                )

                # col path: max over rb (2-step pairwise), then partition max
                qm2a = wpool.tile([128, 1024], f16, tag="qm2a", name=f"qm2a{bc}")
                nc.vector.scalar_tensor_tensor(
                    qm2a[:],
                    Q[:, 0:1024],
                    1.0,
                    Q[:, 1024:2048],
                    op0=ALU.mult,
                    op1=ALU.max,
                )
                qm2 = wpool.tile([128, 512], f16, tag="qm2", name=f"qm2_{bc}")
                nc.vector.scalar_tensor_tensor(
                    qm2[:],
                    qm2a[:, 0:512],
                    1.0,
                    qm2a[:, 512:1024],
                    op0=ALU.mult,
                    op1=ALU.max,
                )
                # partition max on Pool (output must start at partition 0),
                # then min over j on Pool too (XYZWC) -> scalar mrow[0, bc]
                cred = wpool.tile([1, 512], f16, tag="cred", name=f"cred{bc}")
                nc.gpsimd.tensor_reduce(
                    cred[:], qm2[:], axis=AxisListType.C, op=ALU.max
                )
                # cross-lane reduce is max-only: negate on DVE, then Pool
                # XYZWC max computes -(min_j max_i Q); host flips the sign
                ncred = wpool.tile([1, 512], f16, tag="ncred", name=f"ncred{bc}")
                nc.vector.scalar_tensor_tensor(
                    ncred[:], cred[:], -1.0, minf[:], op0=ALU.mult, op1=ALU.max
                )
                nc.gpsimd.tensor_reduce(
                    mrow[0:1, bc : bc + 1],
                    ncred[:],
                    axis=AxisListType.XYZWC,
                    op=ALU.max,
                )

            nc.sync.dma_start(res_d[:], res[:])
            nc.sync.dma_start(mr_d[:], mrow[:])
    nc.finalize()
    return nc


def _enc3(v):
    """Encode v (fp64) as three fp8 arrays summing to ~v."""
    hi = v.astype(np.float32).astype(np8)
    r1 = v - hi.astype(np.float64)
    lo = r1.astype(np.float32).astype(np8)
    r2 = r1 - lo.astype(np.float64)
    lo2 = r2.astype(np.float32).astype(np8)
    return hi, lo, lo2


def _pack_xy(a8):
    """a8: [512 n, 512 w] fp8 -> [128 p, 2048] with [p, kb*1024+t*512+n]."""
    v = a8.T.reshape(2, 2, 128, 512)          # w = kb*256 + t*128 + p
    return v.transpose(2, 0, 1, 3).reshape(128, 2048)


def kernel(input, target, _stats=None):
    x = np.asarray(input, dtype=np.float32)
    y = np.asarray(target, dtype=np.float32)

    # ---- host: dice term (exact) ----
    xf = x.reshape(B, -1).astype(np.float64)
    yf = y.reshape(B, -1).astype(np.float64)
    inter = (xf * yf).sum(axis=1)
    union = xf.sum(axis=1) + yf.sum(axis=1)
    dice = float(np.mean(1.0 - (2.0 * inter + SMOOTH) / (union + SMOOTH)))

    # ---- host: fp8 rounding + layout prep ----
    x8 = x.reshape(B * C, H, W).astype(np8)   # [96, n, w]
    y8 = y.reshape(B * C, H, W).astype(np8)
    x2 = (x8.astype(np.float64) ** 2).sum(axis=2)  # norms of ROUNDED values
    y2 = (y8.astype(np.float64) ** 2).sum(axis=2)
    xh, xl, xl2 = _enc3(-0.5 * x2)            # [96, 512] fp8 each
    yh, yl, yl2 = _enc3(-0.5 * y2)

    in_maps = []
    for c in range(NCORES):
        g0 = c * NBC
        xy8 = np.empty((NBC, 2, 128, 2048), dtype=np8)
        xa8 = np.zeros((3, NBC * 4 * 256), dtype=np8)
        ya8 = np.zeros((3, NBC * 1024), dtype=np8)
        one = np8(1.0)
        for bc in range(NBC):
            g = g0 + bc
            xy8[bc, 0] = _pack_xy(x8[g])
            xy8[bc, 1] = _pack_xy(y8[g])
            for rb in range(4):
                o = (4 * bc + rb) * 256
                i0 = 128 * rb
                # x-side aug slots: (0,0)=x2hi (0,1)=x2lo (1,0)=x2lo2,
                # (1,1)(2,0)(2,1) = ones
                xa8[0, o : o + 128] = xh[g, i0 : i0 + 128]
                xa8[0, o + 128 : o + 256] = xl[g, i0 : i0 + 128]
                xa8[1, o : o + 128] = xl2[g, i0 : i0 + 128]
                xa8[1, o + 128 : o + 256] = one
                xa8[2, o : o + 256] = one
            o = 1024 * bc
            # y-side aug slots: (0,0)(0,1)(1,0) = ones,
            # (1,1)=y2hi (2,0)=y2lo (2,1)=y2lo2
            ya8[0, o : o + 1024] = one
            ya8[1, o : o + 512] = one
            ya8[1, o + 512 : o + 1024] = yh[g]
            ya8[2, o : o + 512] = yl[g]
            ya8[2, o + 512 : o + 1024] = yl2[g]
        in_maps.append({"xy8": xy8, "xa8": xa8, "ya8": ya8})

    if "nc" not in _CACHE:
        _CACHE["nc"] = _build()
    nc = _CACHE["nc"]

    import time as _time

    t0 = _time.time()
    br = run_bass_kernel_spmd(nc, in_maps, list(range(NCORES)), trace=False)
    t1 = _time.time()
    if isinstance(_stats, dict):
        _stats["wall_s"] = t1 - t0
        reps = _stats.get("repeats", 0)
        times = []
        for _ in range(reps):
            ta = _time.time()
            br = run_bass_kernel_spmd(nc, in_maps, list(range(NCORES)), trace=False)
            times.append(_time.time() - ta)
        _stats["repeat_wall_s"] = times

    # ---- host: combine ----
    hds = []
    for c in range(NCORES):
        res = br.results[c]["res"]            # [128, 16]
        rrow = res[:, 0:NBC]                  # min over rb of max_j Q
        mr = -br.results[c]["mr"][0, 0:NBC]   # min_j max_i Q (negated out)
        dxy2 = -2.0 * rrow.min(axis=0)
        dyx2 = -2.0 * mr
        hd2 = np.maximum(np.maximum(dxy2, dyx2), 0.0)
        hds.append(np.sqrt(hd2))
    hd = float(np.mean(np.concatenate(hds)))

    loss = WEIGHT_DICE * dice + WEIGHT_HAUSDORFF * hd
    return np.float32(loss)
